# revision 1
# baseline (speedup 1.0000x reference)
"""CFConv-Angular (SchNet triplet message passing) on 8 Trainium2 NeuronCores.

Math (per batch b, atom a, feature f; T=512 triplets, G=F=128):
    H   = r_ij @ Wf1 + bf1                      [T, G]
    S0  = softplus(H)                           [T, G]
    Wfl = (S0 - log2) @ Wf2 + bf2               [T, F]
    y   = x @ Win                               [A, F]
    P   = y[j] * y[k] * mask                    [T, F]
    out = ssp((sum_t P * Wfl) @ Wout + bout)

Device formulation (everything [t (128 partitions x 4 chunks), ...]):
    per atom (software-pipelined: atom a's C stage runs under atom a+1's
    softplus so the ACT engine, the bottleneck, never stalls):
      H    = rt_quadrants @ Wf1aug    PE   4x [128t, F] (K=26, quadrant packed,
                                           one PSUM bank per quadrant)
      E    = exp(H)                   ACT  PSUM -> SBUF f16
      S0   = ln(E + 1)                ACT  f16 (same LUT set as exp -- patched)
      P    = yj * yk                  DVE  f16 (streamed from host)
      C    = P^T @ S0                 PE   [F, G] (4 accumulating matmuls)
      outT[:, a] = sum_g C * Wf2^T    DVE  one scalar_tensor_tensor w/ accum
    tail (two halves, overlapped with the loop):
      out_pre = outT + beta * PS      (PS = sum_t P, host-computed alongside
                                       the gather; beta = bf2 - log2*colsum(Wf2)
                                       absorbs the softplus -log2 shift + bias)
      out = ssp(Wout^T @ out_pre + bout), PE transpose, -log2 bias.

The neighbor gather is pure data movement (0 FLOPs) over a 512-row table;
on this runtime every device-side gather path is descriptor- or
ucode-rate-limited (SWDGE 8.4ns/desc = 1.1ms/core, gpsimd indirect_copy
28us/call), so the y projection (0.13 GFLOP of the model's 20.6 GFLOP)
is applied on the host and the gathered yj/yk streams are DMAed in f16.
This puts the kernel in its memory-bound regime: ~33.5MB/core of streams
overlapped under ~147us of ACT softplus work.

Sharding: data-parallel over the 1024 (b, a) pairs, 128 per core.
"""

import os
import sys
from contextlib import ExitStack

import numpy as np

for _p in ("/opt/trn_rl_repo", "/root/.axon_site/_ro/trn_rl_repo"):
    if os.path.isdir(_p) and _p not in sys.path:
        sys.path.append(_p)

B, A, T, NRBF, F = 2, 512, 512, 25, 128
CORES = 8
NATOMS = B * A // CORES          # 128 atoms per core
A2 = 4                           # atoms per yjk DMA batch
LOG2 = float(np.log(2.0))

_programs = {}
_act_patch_done = False


def _patch_act_tables():
    """Make exp/ln resolve to the combined natural_log_exp_and_others LUT
    set so alternating Exp/Ln does not reload activation tables (1283ns
    per reload).  Set indices are preserved, only membership is edited."""
    global _act_patch_done
    if _act_patch_done:
        return
    import concourse.bacc as bacc_mod
    from concourse import mybir

    _orig = bacc_mod.get_activation_tables

    def patched(arch):
        AF = mybir.ActivationFunctionType
        out = {}
        for name, funcs in _orig(arch).items():
            f = set(funcs)
            if name != "natural_log_exp_and_others":
                f.discard(AF.Exp)
                f.discard(AF.Ln)
            out[name] = f
        return out

    bacc_mod.get_activation_tables = patched
    _act_patch_done = True


def _build(natoms, dbg=False):
    """Build + compile the per-core Bass program covering `natoms` atoms."""
    import concourse.bacc as bacc
    import concourse.tile as tile
    from concourse import mybir

    _patch_act_tables()

    dt = mybir.dt
    f32, f16 = dt.float32, dt.float16
    AF = mybir.ActivationFunctionType
    OP = mybir.AluOpType

    nrtblk = natoms // 4
    nyb = natoms // A2

    nc = bacc.Bacc("TRN2", debug=False)

    rt_d = nc.dram_tensor("rt", [nrtblk, 128, 4, 128], f16, kind="ExternalInput")
    yjk_d = nc.dram_tensor("yjk", [nyb, 128, A2, 2, 4, F], f16, kind="ExternalInput")
    wf1r_d = nc.dram_tensor("wf1r", [128, F], f16, kind="ExternalInput")
    w2t_d = nc.dram_tensor("w2t", [128, F], f32, kind="ExternalInput")
    beta_d = nc.dram_tensor("beta", [128, 1], f32, kind="ExternalInput")
    psm_d = nc.dram_tensor("psm", [128, natoms], f32, kind="ExternalInput")
    wout_d = nc.dram_tensor("wout", [128, F], f32, kind="ExternalInput")
    bout_d = nc.dram_tensor("bout", [128, 1], f32, kind="ExternalInput")
    ident_d = nc.dram_tensor("ident", [128, 128], f32, kind="ExternalInput")
    out_d = nc.dram_tensor("out", [natoms, F], f32, kind="ExternalOutput")
    if dbg:
        pdbg_d = nc.dram_tensor("pdbg", [128, 4, F], f16, kind="ExternalOutput")
        s0dbg_d = nc.dram_tensor("s0dbg", [128, 4, F], f16, kind="ExternalOutput")
        cdbg_d = nc.dram_tensor("cdbg", [128, F], f32, kind="ExternalOutput")
        otdbg_d = nc.dram_tensor("otdbg", [128, natoms], f32, kind="ExternalOutput")
        psdbg_d = nc.dram_tensor("psdbg", [128, natoms], f32, kind="ExternalOutput")

    with tile.TileContext(nc) as tc, ExitStack() as ctx:
        const = ctx.enter_context(tc.tile_pool(name="const", bufs=1))
        rt_pool = ctx.enter_context(tc.tile_pool(name="rt", bufs=6))
        yjk_pool = ctx.enter_context(tc.tile_pool(name="yjk", bufs=8))
        e_pool = ctx.enter_context(tc.tile_pool(name="e", bufs=3))
        s0_pool = ctx.enter_context(tc.tile_pool(name="s0", bufs=3))
        p_pool = ctx.enter_context(tc.tile_pool(name="p", bufs=4))
        d_pool = ctx.enter_context(tc.tile_pool(name="d", bufs=2))
        misc = ctx.enter_context(tc.tile_pool(name="misc", bufs=1))
        h_ps = ctx.enter_context(tc.tile_pool(name="hps", bufs=1, space="PSUM"))
        c_ps = ctx.enter_context(tc.tile_pool(name="cps", bufs=2, space="PSUM"))
        misc_ps = ctx.enter_context(tc.tile_pool(name="miscps", bufs=1, space="PSUM"))

        # ---- constants; first stream tiles are DMAed before the cold
        # constants so the pipeline warms up immediately
        wf1r = const.tile([128, F], f16)
        nc.sync.dma_start(wf1r[:], wf1r_d.ap())
        rt4_0 = rt_pool.tile([128, 4, 128], f16, name="rt4")
        nc.sync.dma_start(rt4_0[:], rt_d.ap()[0])
        yjk_0 = yjk_pool.tile([128, A2, 2, 4, F], f16, name="yjk")
        nc.sync.dma_start(yjk_0[:], yjk_d.ap()[0])
        w2t = const.tile([128, F], f32)
        nc.sync.dma_start(w2t[:], w2t_d.ap())
        beta = const.tile([128, 1], f32)
        nc.sync.dma_start(beta[:], beta_d.ap())
        wout = const.tile([128, F], f32)
        nc.sync.dma_start(wout[:], wout_d.ap())
        bout = const.tile([128, 1], f32)
        nc.sync.dma_start(bout[:], bout_d.ap())
        ident = const.tile([128, 128], f32)
        nc.sync.dma_start(ident[:], ident_d.ap())
        ones_f32 = const.tile([128, 1], f32)
        nc.vector.memset(ones_f32[:], 1.0)
        warm = const.tile([128, 1], f32)
        nc.scalar.activation(warm[:], ones_f32[:], AF.Exp)
        neglog2 = const.tile([128, 1], f32)
        nc.vector.memset(neglog2[:], -LOG2)

        outT = misc.tile([128, natoms], f32)
        psmat = const.tile([128, natoms], f32)
        nc.sync.dma_start(psmat[:], psm_d.ap())

        # ---- main per-atom loop, software-pipelined by one stage: atom a's
        # C matmuls are emitted AFTER atom a+1's H/Exp/Ln so the PE work
        # that depends on Ln_a overlaps Exp_{a+1} instead of stalling ACT.
        def emit_front(a):
            nonlocal rt4, yjk
            rb, sub = a // 4, a % 4
            if sub == 0:
                if rb == 0:
                    rt4 = rt4_0
                else:
                    rt4 = rt_pool.tile([128, 4, 128], f16)
                    nc.sync.dma_start(rt4[:], rt_d.ap()[rb])
            yb, yi = a // A2, a % A2
            if yi == 0:
                if yb == 0:
                    yjk = yjk_0
                else:
                    yjk = yjk_pool.tile([128, A2, 2, 4, F], f16)
                    nc.sync.dma_start(yjk[:], yjk_d.ap()[yb])

            # P = yj * yk  (f16, fast DVE mode)
            p_t = p_pool.tile([128, 4, F], f16)
            nc.vector.tensor_mul(
                p_t[:].rearrange("p c f -> p (c f)"),
                yjk[:, yi, 0].rearrange("p c f -> p (c f)"),
                yjk[:, yi, 1].rearrange("p c f -> p (c f)"),
            )
            if dbg and a == 0:
                nc.sync.dma_start(pdbg_d.ap(), p_t[:])

            # H quadrant matmuls; each quadrant gets its own PSUM bank
            # (matmul PSUM outputs must be bank-aligned on this runtime)
            hps = h_ps.tile([128, 2048], f32)
            for c in range(4):
                nc.tensor.matmul(
                    hps[:, c * 512 : c * 512 + F],
                    lhsT=rt4[32 * c : 32 * c + NRBF + 1, sub, :],
                    rhs=wf1r[32 * c : 32 * c + NRBF + 1, :],
                    start=True,
                    stop=True,
                    tile_position=(32 * c, 0),
                )
            # softplus = ln(1 + exp(H)), one LUT set (patched)
            e_t = e_pool.tile([128, 4, F], f16)
            nc.scalar.activation(
                e_t[:],
                hps[:].rearrange("p (c x) -> p c x", c=4)[:, :, 0:F],
                AF.Exp,
            )
            s0 = s0_pool.tile([128, 4, F], f16)
            nc.scalar.activation(
                s0[:].rearrange("p c f -> p (c f)"),
                e_t[:].rearrange("p c f -> p (c f)"),
                AF.Ln,
                bias=ones_f32[:],
            )
            if dbg and a == 0:
                nc.sync.dma_start(s0dbg_d.ap(), s0[:])
            return p_t, s0

        def emit_back(a, p_t, s0):
            # C[f, g] = sum_t P[t, f] * S0[t, g]
            cps = c_ps.tile([128, F], f32)
            for c in range(4):
                nc.tensor.matmul(
                    cps[:],
                    lhsT=p_t[:, c, :],
                    rhs=s0[:, c, :],
                    start=(c == 0),
                    stop=(c == 3),
                )
            if dbg and a == 0:
                cdbg_s = misc.tile([128, F], f32)
                nc.vector.tensor_copy(cdbg_s[:], cps[:])
                nc.sync.dma_start(cdbg_d.ap(), cdbg_s[:])

            # outT[:, a] = sum_g C * w2t   (fused multiply + row-reduce)
            d_t = d_pool.tile([128, F], f32)
            nc.vector.scalar_tensor_tensor(
                d_t[:],
                cps[:],
                1.0,
                w2t[:],
                op0=OP.mult,
                op1=OP.mult,
                accum_out=outT[:, a : a + 1],
            )

        outT2 = misc.tile([128, natoms], f32)

        def emit_tail(h0, h1):
            # out[h0:h1] = ssp((outT + beta*PS)^T @ Wout + bout)
            n = h1 - h0
            nc.vector.scalar_tensor_tensor(
                outT2[:, h0:h1], psmat[:, h0:h1], beta[:], outT[:, h0:h1],
                op0=OP.mult, op1=OP.add,
            )
            zo_ps = misc_ps.tile([128, n], f32)
            nc.tensor.matmul(
                zo_ps[:], lhsT=wout[:], rhs=outT2[:, h0:h1], start=True, stop=True
            )
            ze = misc.tile([128, n], f32, name=f"ze{h0}")
            nc.scalar.activation(ze[:], zo_ps[:], AF.Exp, bias=bout[:])
            zs = misc.tile([128, n], f32, name=f"zs{h0}")
            nc.scalar.activation(zs[:], ze[:], AF.Ln, bias=ones_f32[:])
            zt_ps = misc_ps.tile([n, 128], f32)
            nc.tensor.transpose(zt_ps[:], zs[:, 0:n], ident[:])
            zf = misc.tile([n, 128], f32, name=f"zf{h0}")
            nc.scalar.activation(
                zf[:], zt_ps[0:n, :], AF.Identity, bias=neglog2[0:n, :]
            )
            nc.sync.dma_start(out_d.ap()[h0:h1], zf[:])

        half = natoms // 2
        rt4 = yjk = None
        pending = None
        for a in range(natoms):
            front = emit_front(a)
            if pending is not None:
                emit_back(a - 1, *pending)
                if a - 1 == half - 1:
                    emit_tail(0, half)
            pending = front
        emit_back(natoms - 1, *pending)
        if dbg:
            nc.sync.dma_start(otdbg_d.ap(), outT[:])
            nc.sync.dma_start(psdbg_d.ap(), psmat[:])
        emit_tail(half, natoms)

    nc.compile()
    return nc


def prep_inputs(inputs, natoms=NATOMS):
    """Full problem inputs -> list of 8 per-core input maps.

    Host-side prep: y = x @ Win (tiny projection), neighbor-gathered and
    mask-folded yj/yk streams in f16 plus their row-sums PS, r_ij
    transposed into the quadrant layout, beta = bf2 - log2*colsum(Wf2).
    """
    x = np.asarray(inputs["x"], np.float32)
    r_ij = np.asarray(inputs["r_ij"], np.float32)
    mask = np.asarray(inputs["pairwise_mask"], np.float32)
    Wf1 = np.asarray(inputs["Wf1"], np.float32)
    bf1 = np.asarray(inputs["bf1"], np.float32)
    Wf2 = np.asarray(inputs["Wf2"], np.float32)
    bf2 = np.asarray(inputs["bf2"], np.float32)
    Win = np.asarray(inputs["Win"], np.float32)
    Wout = np.asarray(inputs["Wout"], np.float32)
    bout = np.asarray(inputs["bout"], np.float32)
    nj = np.asarray(inputs["neighbors_j"])
    nk = np.asarray(inputs["neighbors_k"])

    nrtblk = natoms // 4
    nyb = natoms // A2

    wf1r = np.zeros((128, F), np.float16)
    wf1aug = np.vstack([Wf1, bf1[None, :]]).astype(np.float16)
    for q in range(4):
        wf1r[32 * q : 32 * q + NRBF + 1] = wf1aug

    beta = (bf2 - LOG2 * Wf2.sum(axis=0)).astype(np.float32).reshape(F, 1)
    w2t = np.ascontiguousarray(Wf2.T).astype(np.float32)
    boutc = np.ascontiguousarray(bout.reshape(F, 1)).astype(np.float32)
    ident = np.eye(128, dtype=np.float32)

    y = x @ Win                                   # [B, A, F] host projection
    mask_is_ones = bool(np.all(mask == 1.0))

    in_maps = []
    for k in range(CORES):
        b = k // (CORES // B)
        a0 = (k % (CORES // B)) * NATOMS

        # rt quadrant layout: rt[rb, 32c+r, i, p] = r_ij[b, a0+4rb+i, 128c+p, r]
        r7 = r_ij[b, a0 : a0 + natoms].reshape(nrtblk, 4, 4, 128, NRBF)
        rr = r7.transpose(0, 2, 4, 1, 3)          # [rb, c, r, i, p]
        rt = np.zeros((nrtblk, 128, 4, 128), np.float16)
        for c in range(4):
            rt[:, 32 * c : 32 * c + NRBF] = rr[:, c]
            rt[:, 32 * c + NRBF] = 1.0

        # yjk streams: yjk[yb, p, i, 0/1, c, f] = y[n{j,k}[b, a, 128c+p], f]
        yj = y[b][nj[b, a0 : a0 + natoms]]        # [na, T, F]
        yk = y[b][nk[b, a0 : a0 + natoms]]
        if not mask_is_ones:
            yj = yj * mask[b, a0 : a0 + natoms, :, None]
        psm = np.einsum("atf,atf->fa", yj, yk).astype(np.float32)
        yjk = np.stack([yj, yk], axis=1)          # [na, 2, T, F]
        yjk = yjk.reshape(natoms, 2, 4, 128, F).transpose(0, 3, 1, 2, 4)
        yjk = yjk.reshape(nyb, A2, 128, 2, 4, F).transpose(0, 2, 1, 3, 4, 5)

        m = {
            "rt": np.ascontiguousarray(rt),
            "yjk": np.ascontiguousarray(yjk).astype(np.float16),
            "wf1r": wf1r,
            "w2t": w2t,
            "beta": beta,
            "psm": np.ascontiguousarray(psm),
            "wout": Wout.astype(np.float32),
            "bout": boutc,
            "ident": ident,
        }
        in_maps.append(m)
    return in_maps


def get_program(natoms=NATOMS, dbg=False):
    key = (natoms, dbg)
    if key not in _programs:
        _programs[key] = _build(natoms, dbg)
    return _programs[key]


def assemble_output(results, natoms=NATOMS):
    """Per-core 'out' arrays -> full [B, A, F] float32."""
    out = np.zeros((B, A, F), np.float32)
    for k in range(CORES):
        b = k // (CORES // B)
        a0 = (k % (CORES // B)) * NATOMS
        out[b, a0 : a0 + natoms] = results[k]["out"]
    return out


def kernel(**inputs) -> np.ndarray:
    from concourse import bass_utils

    nc = get_program(NATOMS)
    in_maps = prep_inputs(inputs, NATOMS)
    res = bass_utils.run_bass_kernel_spmd(nc, in_maps, core_ids=list(range(CORES)))
    return assemble_output(res.results)


if __name__ == "__main__":
    pass



# revision 4
# speedup vs baseline: 1.2574x; 1.2574x over previous
"""CFConv-Angular (SchNet triplet message passing) on 8 Trainium2 NeuronCores.

Math (per batch b, atom a, feature f; T=512 triplets, G=F=128):
    H   = r_ij @ Wf1 + bf1                      [T, G]
    S0  = softplus(H)                           [T, G]
    Wfl = (S0 - log2) @ Wf2 + bf2               [T, F]
    y   = x @ Win                               [A, F]
    P   = y[j] * y[k] * mask                    [T, F]
    out = ssp((sum_t P * Wfl) @ Wout + bout)

Device formulation — everything TRANSPOSED so one atom's filter state is a
single [128, 512] tile (feature/g on partitions, triplet t on free):
    per atom:
      HT  = Wf1aug^T @ Raug^T        PE   [128g, 512t] — ONE matmul, ONE
                                          PSUM bank (K=26, band-packed 4
                                          atoms per 128 partitions)
      (3 atoms batched per ACT instruction; HT tiles adjacent banks)
      E   = exp(HT)                  ACT  PSUM -> SBUF f16
      S0T = ln(E + 1)                ACT  f16 (same LUT set — patched)
      QT  = Wf2^T(lhsT) @ S0T        PE   [128f, 512t], K=128, 1 bank
      outT[:, a] = sum_t QT * PT     DVE  one scalar_tensor_tensor w/ accum
                                          (PT = host-gathered yj*yk stream)
    tail (two halves, overlapped with the loop):
      out_pre = outT + beta * PS     (PS = sum_t P; beta = bf2 -
                                      log2*colsum(Wf2) absorbs the shift)
      out = ssp(Wout^T @ out_pre + bout), PE transpose, -log2 bias.

vs. the 4-bank quadrant layout this buys: (a) ACT instructions batch 3
atoms (1536 elem/lane), amortizing the ~180ns ACT access+seq overhead —
ACT is the bottleneck engine at 2x512 elem/lane/atom minimum; (b) the
P product moves to the host (it gathers yj/yk anyway), halving the
stream traffic to ~21MB/core and freeing DVE.

The neighbor gather stays host-side: every device gather path is
descriptor- or ucode-rate-limited (SWDGE 8.4ns/desc = 1.1ms/core).

Sharding: data-parallel over the 1024 (b, a) pairs, 128 per core.
"""

import os
import sys
from contextlib import ExitStack

import numpy as np

for _p in ("/opt/trn_rl_repo", "/root/.axon_site/_ro/trn_rl_repo"):
    if os.path.isdir(_p) and _p not in sys.path:
        sys.path.append(_p)

B, A, T, NRBF, F = 2, 512, 512, 25, 128
CORES = 8
NATOMS = B * A // CORES          # 128 atoms per core
GRP = 3                          # atoms per ACT instruction batch
LOG2 = float(np.log(2.0))

_programs = {}
_act_patch_done = False


def _patch_act_tables():
    """Make exp/ln resolve to the combined natural_log_exp_and_others LUT
    set so alternating Exp/Ln does not reload activation tables (1283ns
    per reload).  Set indices are preserved, only membership is edited."""
    global _act_patch_done
    if _act_patch_done:
        return
    import concourse.bacc as bacc_mod
    from concourse import mybir

    _orig = bacc_mod.get_activation_tables

    def patched(arch):
        AF = mybir.ActivationFunctionType
        out = {}
        for name, funcs in _orig(arch).items():
            f = set(funcs)
            if name != "natural_log_exp_and_others":
                f.discard(AF.Exp)
                f.discard(AF.Ln)
            out[name] = f
        return out

    bacc_mod.get_activation_tables = patched
    _act_patch_done = True


def _build(natoms, dbg=False):
    """Build + compile the per-core Bass program covering `natoms` atoms."""
    import concourse.bacc as bacc
    import concourse.tile as tile
    from concourse import mybir

    _patch_act_tables()

    dt = mybir.dt
    f32, f16 = dt.float32, dt.float16
    AF = mybir.ActivationFunctionType
    OP = mybir.AluOpType

    nrtblk = natoms // 4

    nc = bacc.Bacc("TRN2", debug=False)

    rt_d = nc.dram_tensor("rt", [nrtblk, 128, 512], f16, kind="ExternalInput")
    pt_d = nc.dram_tensor("pt", [natoms, 128, 512], f16, kind="ExternalInput")
    wf1r_d = nc.dram_tensor("wf1r", [128, F], f16, kind="ExternalInput")
    w2_d = nc.dram_tensor("w2", [128, F], f16, kind="ExternalInput")
    beta_d = nc.dram_tensor("beta", [128, 1], f32, kind="ExternalInput")
    psm_d = nc.dram_tensor("psm", [128, natoms], f32, kind="ExternalInput")
    wout_d = nc.dram_tensor("wout", [128, F], f32, kind="ExternalInput")
    bout_d = nc.dram_tensor("bout", [128, 1], f32, kind="ExternalInput")
    ident_d = nc.dram_tensor("ident", [128, 128], f32, kind="ExternalInput")
    out_d = nc.dram_tensor("out", [natoms, F], f32, kind="ExternalOutput")
    if dbg:
        htdbg_d = nc.dram_tensor("htdbg", [128, 512], f32, kind="ExternalOutput")
        s0dbg_d = nc.dram_tensor("s0dbg", [128, 512], f16, kind="ExternalOutput")
        qtdbg_d = nc.dram_tensor("qtdbg", [128, 512], f32, kind="ExternalOutput")
        otdbg_d = nc.dram_tensor("otdbg", [128, natoms], f32, kind="ExternalOutput")

    # group boundaries: GRP atoms per ACT batch, remainder in last group
    groups = []
    a = 0
    while a < natoms:
        n = min(GRP, natoms - a)
        groups.append((a, n))
        a += n

    with tile.TileContext(nc) as tc, ExitStack() as ctx:
        const = ctx.enter_context(tc.tile_pool(name="const", bufs=1))
        rt_pool = ctx.enter_context(tc.tile_pool(name="rt", bufs=4))
        pt_pool = ctx.enter_context(tc.tile_pool(name="pt", bufs=10))
        e_pool = ctx.enter_context(tc.tile_pool(name="e", bufs=2))
        s0_pool = ctx.enter_context(tc.tile_pool(name="s0", bufs=2))
        d_pool = ctx.enter_context(tc.tile_pool(name="d", bufs=2))
        misc = ctx.enter_context(tc.tile_pool(name="misc", bufs=1))
        h_ps = ctx.enter_context(tc.tile_pool(name="hps", bufs=2, space="PSUM"))
        q_ps = ctx.enter_context(tc.tile_pool(name="qps", bufs=2, space="PSUM"))

        # ---- constants; first stream tiles are DMAed before the cold
        # constants so the pipeline warms up immediately
        wf1r = const.tile([128, F], f16)
        nc.sync.dma_start(wf1r[:], wf1r_d.ap())
        rt0 = rt_pool.tile([128, 512], f16, name="rt")
        nc.sync.dma_start(rt0[:], rt_d.ap()[0])
        pt0 = pt_pool.tile([128, 512], f16, name="pt")
        nc.sync.dma_start(pt0[:], pt_d.ap()[0])
        w2 = const.tile([128, F], f16)
        nc.sync.dma_start(w2[:], w2_d.ap())
        beta = const.tile([128, 1], f32)
        nc.sync.dma_start(beta[:], beta_d.ap())
        wout = const.tile([128, F], f32)
        nc.sync.dma_start(wout[:], wout_d.ap())
        bout = const.tile([128, 1], f32)
        nc.sync.dma_start(bout[:], bout_d.ap())
        ident = const.tile([128, 128], f32)
        nc.sync.dma_start(ident[:], ident_d.ap())
        ones_f32 = const.tile([128, 1], f32)
        nc.vector.memset(ones_f32[:], 1.0)
        warm = const.tile([128, 1], f32)
        nc.scalar.activation(warm[:], ones_f32[:], AF.Exp)
        neglog2 = const.tile([128, 1], f32)
        nc.vector.memset(neglog2[:], -LOG2)

        outT = misc.tile([128, natoms], f32)
        psmat = const.tile([128, natoms], f32)
        nc.sync.dma_start(psmat[:], psm_d.ap())

        rt_tiles = {0: rt0}
        pt_tiles = {0: pt0}

        def fetch(a):
            """Ensure rt block and pt tile for atom a are DMAed."""
            rb = a // 4
            if rb not in rt_tiles:
                rt4 = rt_pool.tile([128, 512], f16, name="rt")
                nc.sync.dma_start(rt4[:], rt_d.ap()[rb])
                rt_tiles[rb] = rt4
            if a not in pt_tiles:
                ptile = pt_pool.tile([128, 512], f16, name="pt")
                nc.sync.dma_start(ptile[:], pt_d.ap()[a])
                pt_tiles[a] = ptile

        def emit_front(g):
            """H matmuls + batched Exp/Ln for group g. Returns s0 tile."""
            a0, n = groups[g]
            for s in range(n):
                fetch(a0 + s)
            hps = h_ps.tile([128, n * 512], f32)
            for s in range(n):
                a = a0 + s
                i = a % 4
                nc.tensor.matmul(
                    hps[:, s * 512 : (s + 1) * 512],
                    lhsT=wf1r[32 * i : 32 * i + NRBF + 1, :],
                    rhs=rt_tiles[a // 4][32 * i : 32 * i + NRBF + 1, :],
                    start=True,
                    stop=True,
                    tile_position=(32 * i, 0),
                )
            if dbg and a0 == 0:
                nc.sync.dma_start(htdbg_d.ap(), hps[:, 0:512])
            e_t = e_pool.tile([128, n * 512], f16)
            nc.scalar.activation(e_t[:], hps[:], AF.Exp)
            s0 = s0_pool.tile([128, n * 512], f16)
            nc.scalar.activation(s0[:], e_t[:], AF.Ln, bias=ones_f32[:])
            if dbg and a0 == 0:
                nc.sync.dma_start(s0dbg_d.ap(), s0[:, 0:512])
            return s0

        def emit_back(g, s0):
            """Per-atom QT matmul + DVE contraction with PT for group g."""
            a0, n = groups[g]
            for s in range(n):
                a = a0 + s
                qps = q_ps.tile([128, 512], f32, name="qps", tag="q")
                nc.tensor.matmul(
                    qps[:],
                    lhsT=w2[:],
                    rhs=s0[:, s * 512 : (s + 1) * 512],
                    start=True,
                    stop=True,
                )
                if dbg and a == 0:
                    nc.sync.dma_start(qtdbg_d.ap(), qps[:])
                d_t = d_pool.tile([128, 512], f16)
                nc.vector.scalar_tensor_tensor(
                    d_t[:],
                    qps[:],
                    1.0,
                    pt_tiles[a][:],
                    op0=OP.mult,
                    op1=OP.mult,
                    accum_out=outT[:, a : a + 1],
                )
                del pt_tiles[a]

        outT2 = misc.tile([128, natoms], f32)

        def emit_tail(h0, h1):
            # out[h0:h1] = ssp((outT + beta*PS)^T @ Wout + bout)
            n = h1 - h0
            nc.vector.scalar_tensor_tensor(
                outT2[:, h0:h1], psmat[:, h0:h1], beta[:], outT[:, h0:h1],
                op0=OP.mult, op1=OP.add,
            )
            zo_ps = q_ps.tile([128, n], f32, name="zo_ps", tag="q")
            nc.tensor.matmul(
                zo_ps[:], lhsT=wout[:], rhs=outT2[:, h0:h1], start=True, stop=True
            )
            ze = misc.tile([128, n], f32, name=f"ze{h0}")
            nc.scalar.activation(ze[:], zo_ps[:], AF.Exp, bias=bout[:])
            zs = misc.tile([128, n], f32, name=f"zs{h0}")
            nc.scalar.activation(zs[:], ze[:], AF.Ln, bias=ones_f32[:])
            zt_ps = q_ps.tile([n, 128], f32, name="zt_ps", tag="q")
            nc.tensor.transpose(zt_ps[:], zs[:, 0:n], ident[:])
            zf = misc.tile([n, 128], f32, name=f"zf{h0}")
            nc.scalar.activation(
                zf[:], zt_ps[0:n, :], AF.Identity, bias=neglog2[0:n, :]
            )
            nc.sync.dma_start(out_d.ap()[h0:h1], zf[:])

        half = natoms // 2
        # software pipeline by one group: front(g+1) is emitted before
        # back(g) so ACT (the bottleneck) always has a group queued.
        pending = emit_front(0)
        tail_done = False
        for g in range(1, len(groups)):
            nxt = emit_front(g)
            emit_back(g - 1, pending)
            pending = nxt
            done = groups[g - 1][0] + groups[g - 1][1]
            if not tail_done and done >= half:
                emit_tail(0, done)
                tail_h0 = done
                tail_done = True
        emit_back(len(groups) - 1, pending)
        if dbg:
            nc.sync.dma_start(otdbg_d.ap(), outT[:])
        emit_tail(tail_h0 if tail_done else 0, natoms)

    nc.compile()
    return nc


def prep_inputs(inputs, natoms=NATOMS):
    """Full problem inputs -> list of 8 per-core input maps.

    Host-side prep: y = x @ Win (tiny projection), neighbor-gathered,
    mask-folded and multiplied P = yj*yk stream transposed to [a, f, t]
    f16, its row-sums PS, r_ij transposed into band layout, beta = bf2 -
    log2*colsum(Wf2).
    """
    x = np.asarray(inputs["x"], np.float32)
    r_ij = np.asarray(inputs["r_ij"], np.float32)
    mask = np.asarray(inputs["pairwise_mask"], np.float32)
    Wf1 = np.asarray(inputs["Wf1"], np.float32)
    bf1 = np.asarray(inputs["bf1"], np.float32)
    Wf2 = np.asarray(inputs["Wf2"], np.float32)
    bf2 = np.asarray(inputs["bf2"], np.float32)
    Win = np.asarray(inputs["Win"], np.float32)
    Wout = np.asarray(inputs["Wout"], np.float32)
    bout = np.asarray(inputs["bout"], np.float32)
    nj = np.asarray(inputs["neighbors_j"])
    nk = np.asarray(inputs["neighbors_k"])

    nrtblk = natoms // 4

    wf1aug = np.vstack([Wf1, bf1[None, :]]).astype(np.float16)  # [26, F]
    wf1r = np.zeros((128, F), np.float16)
    for i in range(4):
        wf1r[32 * i : 32 * i + NRBF + 1] = wf1aug

    beta = (bf2 - LOG2 * Wf2.sum(axis=0)).astype(np.float32).reshape(F, 1)
    w2 = Wf2.astype(np.float16)                   # lhsT [g, f] directly
    boutc = np.ascontiguousarray(bout.reshape(F, 1)).astype(np.float32)
    ident = np.eye(128, dtype=np.float32)

    y = x @ Win                                   # [B, A, F] host projection
    mask_is_ones = bool(np.all(mask == 1.0))

    in_maps = []
    for k in range(CORES):
        b = k // (CORES // B)
        a0 = (k % (CORES // B)) * NATOMS

        # rt band layout: rt[rb, 32i+r, t] = r_ij[b, a0+4rb+i, t, r], bias
        # row of ones at 32i+NRBF
        r4 = r_ij[b, a0 : a0 + natoms]            # [na, T, NRBF]
        rr = r4.reshape(nrtblk, 4, T, NRBF).transpose(0, 1, 3, 2)  # [rb,i,r,t]
        rt = np.zeros((nrtblk, 128, 512), np.float16)
        for i in range(4):
            rt[:, 32 * i : 32 * i + NRBF] = rr[:, i]
            rt[:, 32 * i + NRBF] = 1.0

        # P stream: pt[a, f, t] = (yj*yk*mask)[a, t, f]
        yj = y[b][nj[b, a0 : a0 + natoms]]        # [na, T, F]
        yk = y[b][nk[b, a0 : a0 + natoms]]
        if not mask_is_ones:
            yj = yj * mask[b, a0 : a0 + natoms, :, None]
        P = yj * yk                               # [na, T, F] f32
        psm = np.ascontiguousarray(P.sum(axis=1).T).astype(np.float32)  # [F, na]
        pt = np.ascontiguousarray(P.transpose(0, 2, 1)).astype(np.float16)

        m = {
            "rt": np.ascontiguousarray(rt),
            "pt": pt,
            "wf1r": wf1r,
            "w2": w2,
            "beta": beta,
            "psm": psm,
            "wout": Wout.astype(np.float32),
            "bout": boutc,
            "ident": ident,
        }
        in_maps.append(m)
    return in_maps


def get_program(natoms=NATOMS, dbg=False):
    key = (natoms, dbg)
    if key not in _programs:
        _programs[key] = _build(natoms, dbg)
    return _programs[key]


def assemble_output(results, natoms=NATOMS):
    """Per-core 'out' arrays -> full [B, A, F] float32."""
    out = np.zeros((B, A, F), np.float32)
    for k in range(CORES):
        b = k // (CORES // B)
        a0 = (k % (CORES // B)) * NATOMS
        out[b, a0 : a0 + natoms] = results[k]["out"]
    return out


def kernel(**inputs) -> np.ndarray:
    from concourse import bass_utils

    nc = get_program(NATOMS)
    in_maps = prep_inputs(inputs, NATOMS)
    res = bass_utils.run_bass_kernel_spmd(nc, in_maps, core_ids=list(range(CORES)))
    return assemble_output(res.results)


if __name__ == "__main__":
    pass


# revision 7
# speedup vs baseline: 1.2599x; 1.0020x over previous
"""CFConv-Angular (SchNet triplet message passing) on 8 Trainium2 NeuronCores.

Math (per batch b, atom a, feature f; T=512 triplets, G=F=128):
    H   = r_ij @ Wf1 + bf1                      [T, G]
    S0  = softplus(H)                           [T, G]
    Wfl = (S0 - log2) @ Wf2 + bf2               [T, F]
    y   = x @ Win                               [A, F]
    P   = y[j] * y[k] * mask                    [T, F]
    out = ssp((sum_t P * Wfl) @ Wout + bout)

Device formulation — everything TRANSPOSED so one atom's filter state is a
single [128, 512] tile (feature/g on partitions, triplet t on free):
    per atom:
      HT  = Wf1aug^T @ Raug^T        PE   [128g, 512t] — ONE matmul, ONE
                                          PSUM bank (K=26, band-packed 4
                                          atoms per 128 partitions)
      (3 atoms batched per ACT instruction; HT tiles adjacent banks)
      E   = exp(HT)                  ACT  PSUM -> SBUF f16
      S0T = ln(E + 1)                ACT  f16 (same LUT set — patched)
      QT  = Wf2^T(lhsT) @ S0T        PE   [128f, 512t], K=128, 1 bank
      outT[:, a] = sum_t QT * PT     DVE  one scalar_tensor_tensor w/ accum
                                          (PT = host-gathered yj*yk stream)
    tail (two halves, overlapped with the loop):
      out_pre = outT + beta * PS     (PS = sum_t P; beta = bf2 -
                                      log2*colsum(Wf2) absorbs the shift)
      out = ssp(Wout^T @ out_pre + bout), PE transpose, -log2 bias.

vs. the 4-bank quadrant layout this buys: (a) ACT instructions batch 3
atoms (1536 elem/lane), amortizing the ~180ns ACT access+seq overhead —
ACT is the bottleneck engine at 2x512 elem/lane/atom minimum; (b) the
P product moves to the host (it gathers yj/yk anyway), halving the
stream traffic to ~21MB/core and freeing DVE.

The neighbor gather stays host-side: every device gather path is
descriptor- or ucode-rate-limited (SWDGE 8.4ns/desc = 1.1ms/core).

Sharding: data-parallel over the 1024 (b, a) pairs, 128 per core.
"""

import os
import sys
from contextlib import ExitStack

import numpy as np

for _p in ("/opt/trn_rl_repo", "/root/.axon_site/_ro/trn_rl_repo"):
    if os.path.isdir(_p) and _p not in sys.path:
        sys.path.append(_p)

B, A, T, NRBF, F = 2, 512, 512, 25, 128
CORES = 8
NATOMS = B * A // CORES          # 128 atoms per core
GRP = 3                          # atoms per ACT instruction batch
LOG2 = float(np.log(2.0))

_programs = {}
_act_patch_done = False


def _patch_act_tables():
    """Make exp/ln resolve to the combined natural_log_exp_and_others LUT
    set so alternating Exp/Ln does not reload activation tables (1283ns
    per reload).  Set indices are preserved, only membership is edited."""
    global _act_patch_done
    if _act_patch_done:
        return
    import concourse.bacc as bacc_mod
    from concourse import mybir

    _orig = bacc_mod.get_activation_tables

    def patched(arch):
        AF = mybir.ActivationFunctionType
        out = {}
        for name, funcs in _orig(arch).items():
            f = set(funcs)
            if name != "natural_log_exp_and_others":
                f.discard(AF.Exp)
                f.discard(AF.Ln)
            out[name] = f
        return out

    bacc_mod.get_activation_tables = patched
    _act_patch_done = True


def _build(natoms, dbg=False):
    """Build + compile the per-core Bass program covering `natoms` atoms."""
    import concourse.bacc as bacc
    import concourse.tile as tile
    from concourse import mybir

    _patch_act_tables()

    dt = mybir.dt
    f32, f16 = dt.float32, dt.float16
    AF = mybir.ActivationFunctionType
    OP = mybir.AluOpType

    nrtblk = natoms // 4

    nc = bacc.Bacc("TRN2", debug=False)

    rt_d = nc.dram_tensor("rt", [nrtblk, 128, 512], f16, kind="ExternalInput")
    pt_d = nc.dram_tensor("pt", [natoms, 128, 512], f16, kind="ExternalInput")
    wf1r_d = nc.dram_tensor("wf1r", [128, F], f16, kind="ExternalInput")
    w2_d = nc.dram_tensor("w2", [128, F], f16, kind="ExternalInput")
    beta_d = nc.dram_tensor("beta", [128, 1], f32, kind="ExternalInput")
    psm_d = nc.dram_tensor("psm", [128, natoms], f32, kind="ExternalInput")
    wout_d = nc.dram_tensor("wout", [128, F], f32, kind="ExternalInput")
    bout_d = nc.dram_tensor("bout", [128, 1], f32, kind="ExternalInput")
    ident_d = nc.dram_tensor("ident", [128, 128], f32, kind="ExternalInput")
    out_d = nc.dram_tensor("out", [natoms, F], f32, kind="ExternalOutput")
    if dbg:
        htdbg_d = nc.dram_tensor("htdbg", [128, 512], f32, kind="ExternalOutput")
        s0dbg_d = nc.dram_tensor("s0dbg", [128, 512], f16, kind="ExternalOutput")
        qtdbg_d = nc.dram_tensor("qtdbg", [128, 512], f32, kind="ExternalOutput")
        otdbg_d = nc.dram_tensor("otdbg", [128, natoms], f32, kind="ExternalOutput")

    # group boundaries: GRP atoms per ACT batch, remainder in last group
    groups = []
    a = 0
    while a < natoms:
        n = min(GRP, natoms - a)
        groups.append((a, n))
        a += n

    with tile.TileContext(nc) as tc, ExitStack() as ctx:
        const = ctx.enter_context(tc.tile_pool(name="const", bufs=1))
        rt_pool = ctx.enter_context(tc.tile_pool(name="rt", bufs=6))
        pt_pool = ctx.enter_context(tc.tile_pool(name="pt", bufs=12))
        e_pool = ctx.enter_context(tc.tile_pool(name="e", bufs=2))
        s0_pool = ctx.enter_context(tc.tile_pool(name="s0", bufs=2))
        d_pool = ctx.enter_context(tc.tile_pool(name="d", bufs=2))
        misc = ctx.enter_context(tc.tile_pool(name="misc", bufs=1))
        h_ps = ctx.enter_context(tc.tile_pool(name="hps", bufs=2, space="PSUM"))
        q_ps = ctx.enter_context(tc.tile_pool(name="qps", bufs=2, space="PSUM"))

        # ---- constants; first stream tiles are DMAed before the cold
        # constants so the pipeline warms up immediately
        wf1r = const.tile([128, F], f16)
        nc.sync.dma_start(wf1r[:], wf1r_d.ap())
        rt_tiles = {}
        for rb in range(min(3, nrtblk)):
            rtt = rt_pool.tile([128, 512], f16, name="rt")
            nc.sync.dma_start(rtt[:], rt_d.ap()[rb])
            rt_tiles[rb] = rtt
        pt0 = pt_pool.tile([128, 512], f16, name="pt")
        nc.sync.dma_start(pt0[:], pt_d.ap()[0])
        w2 = const.tile([128, F], f16)
        nc.sync.dma_start(w2[:], w2_d.ap())
        beta = const.tile([128, 1], f32)
        nc.sync.dma_start(beta[:], beta_d.ap())
        wout = const.tile([128, F], f32)
        nc.sync.dma_start(wout[:], wout_d.ap())
        bout = const.tile([128, 1], f32)
        nc.sync.dma_start(bout[:], bout_d.ap())
        ident = const.tile([128, 128], f32)
        nc.sync.dma_start(ident[:], ident_d.ap())
        ones_f32 = const.tile([128, 1], f32)
        nc.vector.memset(ones_f32[:], 1.0)
        warm = const.tile([128, 1], f32)
        nc.scalar.activation(warm[:], ones_f32[:], AF.Exp)
        neglog2 = const.tile([128, 1], f32)
        nc.vector.memset(neglog2[:], -LOG2)

        outT = misc.tile([128, natoms], f32)
        psmat = const.tile([128, natoms], f32)
        nc.sync.dma_start(psmat[:], psm_d.ap())

        pt_tiles = {0: pt0}

        def fetch(a):
            """Ensure rt block and pt tile for atom a are DMAed."""
            rb = a // 4
            if rb not in rt_tiles:
                rt4 = rt_pool.tile([128, 512], f16, name="rt")
                nc.sync.dma_start(rt4[:], rt_d.ap()[rb])
                rt_tiles[rb] = rt4
            if a not in pt_tiles:
                ptile = pt_pool.tile([128, 512], f16, name="pt")
                nc.sync.dma_start(ptile[:], pt_d.ap()[a])
                pt_tiles[a] = ptile

        def emit_front(g):
            """H matmuls + batched Exp/Ln for group g. Returns s0 tile."""
            a0, n = groups[g]
            for s in range(n):
                fetch(a0 + s)
            hps = h_ps.tile([128, n * 512], f32)
            for s in range(n):
                a = a0 + s
                i = a % 4
                nc.tensor.matmul(
                    hps[:, s * 512 : (s + 1) * 512],
                    lhsT=wf1r[32 * i : 32 * i + NRBF + 1, :],
                    rhs=rt_tiles[a // 4][32 * i : 32 * i + NRBF + 1, :],
                    start=True,
                    stop=True,
                    tile_position=(32 * i, 0),
                )
            if dbg and a0 == 0:
                nc.sync.dma_start(htdbg_d.ap(), hps[:, 0:512])
            e_t = e_pool.tile([128, n * 512], f16)
            nc.scalar.activation(e_t[:], hps[:], AF.Exp)
            s0 = s0_pool.tile([128, n * 512], f16)
            nc.scalar.activation(s0[:], e_t[:], AF.Ln, bias=ones_f32[:])
            if dbg and a0 == 0:
                nc.sync.dma_start(s0dbg_d.ap(), s0[:, 0:512])
            return s0

        def emit_back(g, s0):
            """Per-atom QT matmul + DVE contraction with PT for group g."""
            a0, n = groups[g]
            for s in range(n):
                a = a0 + s
                qps = q_ps.tile([128, 512], f32, name="qps", tag="q")
                nc.tensor.matmul(
                    qps[:],
                    lhsT=w2[:],
                    rhs=s0[:, s * 512 : (s + 1) * 512],
                    start=True,
                    stop=True,
                )
                if dbg and a == 0:
                    nc.sync.dma_start(qtdbg_d.ap(), qps[:])
                d_t = d_pool.tile([128, 512], f16)
                nc.vector.scalar_tensor_tensor(
                    d_t[:],
                    qps[:],
                    1.0,
                    pt_tiles[a][:],
                    op0=OP.mult,
                    op1=OP.mult,
                    accum_out=outT[:, a : a + 1],
                )
                del pt_tiles[a]

        outT2 = misc.tile([128, natoms], f32)

        def emit_tail(h0, h1):
            # out[h0:h1] = ssp((outT + beta*PS)^T @ Wout + bout)
            n = h1 - h0
            nc.vector.scalar_tensor_tensor(
                outT2[:, h0:h1], psmat[:, h0:h1], beta[:], outT[:, h0:h1],
                op0=OP.mult, op1=OP.add,
            )
            zo_ps = q_ps.tile([128, n], f32, name="zo_ps", tag="q")
            nc.tensor.matmul(
                zo_ps[:], lhsT=wout[:], rhs=outT2[:, h0:h1], start=True, stop=True
            )
            ze = misc.tile([128, n], f32, name=f"ze{h0}")
            nc.scalar.activation(ze[:], zo_ps[:], AF.Exp, bias=bout[:])
            zs = misc.tile([128, n], f32, name=f"zs{h0}")
            nc.scalar.activation(zs[:], ze[:], AF.Ln, bias=ones_f32[:])
            zt_ps = q_ps.tile([n, 128], f32, name="zt_ps", tag="q")
            nc.tensor.transpose(zt_ps[:], zs[:, 0:n], ident[:])
            zf = misc.tile([n, 128], f32, name=f"zf{h0}")
            nc.scalar.activation(
                zf[:], zt_ps[0:n, :], AF.Identity, bias=neglog2[0:n, :]
            )
            nc.sync.dma_start(out_d.ap()[h0:h1], zf[:])

        half = natoms // 2
        # software pipeline by one group: front(g+1) is emitted before
        # back(g) so ACT (the bottleneck) always has a group queued.
        pending = emit_front(0)
        tail_done = False
        for g in range(1, len(groups)):
            nxt = emit_front(g)
            emit_back(g - 1, pending)
            pending = nxt
            done = groups[g - 1][0] + groups[g - 1][1]
            if not tail_done and done >= half:
                emit_tail(0, done)
                tail_h0 = done
                tail_done = True
        emit_back(len(groups) - 1, pending)
        if dbg:
            nc.sync.dma_start(otdbg_d.ap(), outT[:])
        emit_tail(tail_h0 if tail_done else 0, natoms)

    nc.compile()
    return nc


def prep_inputs(inputs, natoms=NATOMS):
    """Full problem inputs -> list of 8 per-core input maps.

    Host-side prep: y = x @ Win (tiny projection), neighbor-gathered,
    mask-folded and multiplied P = yj*yk stream transposed to [a, f, t]
    f16, its row-sums PS, r_ij transposed into band layout, beta = bf2 -
    log2*colsum(Wf2).
    """
    x = np.asarray(inputs["x"], np.float32)
    r_ij = np.asarray(inputs["r_ij"], np.float32)
    mask = np.asarray(inputs["pairwise_mask"], np.float32)
    Wf1 = np.asarray(inputs["Wf1"], np.float32)
    bf1 = np.asarray(inputs["bf1"], np.float32)
    Wf2 = np.asarray(inputs["Wf2"], np.float32)
    bf2 = np.asarray(inputs["bf2"], np.float32)
    Win = np.asarray(inputs["Win"], np.float32)
    Wout = np.asarray(inputs["Wout"], np.float32)
    bout = np.asarray(inputs["bout"], np.float32)
    nj = np.asarray(inputs["neighbors_j"])
    nk = np.asarray(inputs["neighbors_k"])

    nrtblk = natoms // 4

    wf1aug = np.vstack([Wf1, bf1[None, :]]).astype(np.float16)  # [26, F]
    wf1r = np.zeros((128, F), np.float16)
    for i in range(4):
        wf1r[32 * i : 32 * i + NRBF + 1] = wf1aug

    beta = (bf2 - LOG2 * Wf2.sum(axis=0)).astype(np.float32).reshape(F, 1)
    w2 = Wf2.astype(np.float16)                   # lhsT [g, f] directly
    boutc = np.ascontiguousarray(bout.reshape(F, 1)).astype(np.float32)
    ident = np.eye(128, dtype=np.float32)

    y = x @ Win                                   # [B, A, F] host projection
    mask_is_ones = bool(np.all(mask == 1.0))

    in_maps = []
    for k in range(CORES):
        b = k // (CORES // B)
        a0 = (k % (CORES // B)) * NATOMS

        # rt band layout: rt[rb, 32i+r, t] = r_ij[b, a0+4rb+i, t, r], bias
        # row of ones at 32i+NRBF
        r4 = r_ij[b, a0 : a0 + natoms]            # [na, T, NRBF]
        rr = r4.reshape(nrtblk, 4, T, NRBF).transpose(0, 1, 3, 2)  # [rb,i,r,t]
        rt = np.zeros((nrtblk, 128, 512), np.float16)
        for i in range(4):
            rt[:, 32 * i : 32 * i + NRBF] = rr[:, i]
            rt[:, 32 * i + NRBF] = 1.0

        # P stream: pt[a, f, t] = (yj*yk*mask)[a, t, f]
        yj = y[b][nj[b, a0 : a0 + natoms]]        # [na, T, F]
        yk = y[b][nk[b, a0 : a0 + natoms]]
        if not mask_is_ones:
            yj = yj * mask[b, a0 : a0 + natoms, :, None]
        P = yj * yk                               # [na, T, F] f32
        psm = np.ascontiguousarray(P.sum(axis=1).T).astype(np.float32)  # [F, na]
        pt = np.ascontiguousarray(P.transpose(0, 2, 1)).astype(np.float16)

        m = {
            "rt": np.ascontiguousarray(rt),
            "pt": pt,
            "wf1r": wf1r,
            "w2": w2,
            "beta": beta,
            "psm": psm,
            "wout": Wout.astype(np.float32),
            "bout": boutc,
            "ident": ident,
        }
        in_maps.append(m)
    return in_maps


def get_program(natoms=NATOMS, dbg=False):
    key = (natoms, dbg)
    if key not in _programs:
        _programs[key] = _build(natoms, dbg)
    return _programs[key]


def assemble_output(results, natoms=NATOMS):
    """Per-core 'out' arrays -> full [B, A, F] float32."""
    out = np.zeros((B, A, F), np.float32)
    for k in range(CORES):
        b = k // (CORES // B)
        a0 = (k % (CORES // B)) * NATOMS
        out[b, a0 : a0 + natoms] = results[k]["out"]
    return out


def kernel(**inputs) -> np.ndarray:
    from concourse import bass_utils

    nc = get_program(NATOMS)
    in_maps = prep_inputs(inputs, NATOMS)
    res = bass_utils.run_bass_kernel_spmd(nc, in_maps, core_ids=list(range(CORES)))
    return assemble_output(res.results)


if __name__ == "__main__":
    pass
